# revision 1
# baseline (speedup 1.0000x reference)
"""AttMaxPool2D (2x2 softmax-attention pooling) Trainium2 Bass kernel.

Problem: x [16, 224, 224, 128] f32 NHWC -> out [16, 112, 112, 128]
  patches = 2x2 non-overlapping windows; out = sum(p * softmax(p, axis=window)).

Sharding: pure data parallel over batch: 8 cores x 2 examples each.

Per-core layout: partition dim = flattened output row (b_loc*112+ho), free dim
= segments of the input row-pair.  Each chunk loads the even row segment and
the odd row segment (fully contiguous per partition -> 2-dim DMA APs with
4KB-contiguous descriptors), computes exp on ACT, then the softmax-weighted
window sum on DVE:
  out = (A*eA + B*eB + C*eC + D*eD) / (eA+eB+eC+eD)
where A,B = (even row, even/odd col), C,D = (odd row, even/odd col).
"""

import os
from contextlib import ExitStack

import numpy as np

import concourse.bass as bass
import concourse.mybir as mybir
import concourse.tile as tile

F32 = mybir.dt.float32

# Full problem shape (hardcoded per contract).
B, H, W, C = 16, 224, 224, 128
N_CORES = 8
B_LOC = B // N_CORES


def _legalize_waits(nc, max_waits=1):
    """This walrus build's ISA structs accept a single sync-wait command per
    instruction, but Tile's wait emission (not transitively minimal) can leave
    2+ waits.  Two-step fix, semantics-preserving:
      1. prune a wait when it is provably dominated through a kept wait
         (some instruction on the kept wait's engine proc, at/before the kept
         wait value, itself directly waits on the dropped semaphore at >= the
         dropped value);
      2. hoist any remaining extras onto same-engine NoOp instructions
         inserted immediately before (sequencer program order preserves the
         blocking semantics)."""
    import bass_rust
    from concourse.tile_scheduler import PROC_NAME_TO_IDX

    f = nc.m.functions[0]
    insts = [i for b in f.blocks for i in b.instructions]

    def pidx(ant_name):
        return PROC_NAME_TO_IDX[ant_name.rsplit("_", 1)[0]]

    by_proc = {}
    for i in insts:
        p = getattr(i, "bass_scheduled_proc", None)
        t = getattr(i, "bass_scheduled_tick", None)
        if p is None or t is None:
            continue
        by_proc.setdefault(p, []).append((t, i))
    for v in by_proc.values():
        v.sort(key=lambda x: x[0])

    def direct_waits(j):
        si = j.sync_info
        out = {}
        for w in si.on_wait if si else []:
            k = pidx(w.ant_name)
            out[k] = max(out.get(k, -1), w.wait_value)
        return out

    engine_procs = {v for k, v in PROC_NAME_TO_IDX.items()
                    if not k.startswith(("DMAHW", "DMASW", "Collectives"))}

    nop_ctr = [0]
    for b in f.blocks:
        new_insts = []
        for i in b.instructions:
            si = i.sync_info
            if not si or len(si.on_wait) <= max_waits:
                new_insts.append(i)
                continue
            # dedupe per-sem (keep max value)
            best = {}
            for w in si.on_wait:
                k = (w.sync_type, w.id)
                if k not in best or w.wait_value > best[k].wait_value:
                    best[k] = w
            kept = list(best.values())
            # drop same-proc self-waits: an engine instruction waiting on its
            # own proc's semaphore for a tick strictly below its own scheduled
            # tick is guaranteed by program order (the engine runs serially);
            # keeping it only stalls on the ~1us deferred sem-write of the
            # predecessor.
            own_p = getattr(i, "bass_scheduled_proc", None)
            own_t = getattr(i, "bass_scheduled_tick", None)
            if own_p is not None and own_t is not None and i.opcode != "DMACopy":
                kept = [w for w in kept
                        if not (pidx(w.ant_name) == own_p
                                and w.wait_value < own_t)]
            # step 1: transitive pruning
            for wd in list(kept):
                if len(kept) <= max_waits:
                    break
                wd_p, wd_v = pidx(wd.ant_name), wd.wait_value
                ok = False
                for via in kept:
                    if via is wd:
                        continue
                    via_p, via_v = pidx(via.ant_name), via.wait_value
                    if via_p not in engine_procs:
                        continue
                    for t, j in by_proc.get(via_p, []):
                        if t > via_v:
                            break
                        if direct_waits(j).get(wd_p, -1) >= wd_v:
                            ok = True
                            break
                    if ok:
                        break
                if ok:
                    kept.remove(wd)
            # step 2: hoist extras onto preceding same-engine NoOps
            while len(kept) > max_waits:
                w = kept.pop(0)
                nop = mybir.InstNoOp(name=f"I-waitnop-{nop_ctr[0]}", ins=[], outs=[])
                nop_ctr[0] += 1
                nop.engine = i.engine
                nop.sync_info = bass_rust.SyncInfo(on_wait=[w], on_update=[])
                new_insts.append(nop)
            si.on_wait = kept
            new_insts.append(i)
        b.instructions = new_insts
    return nc


def build_kernel(b_loc=B_LOC, h=H, w=W, c=C, f=2048, legalize=True):
    """Emit the per-core kernel. f = input-row segment length (elems) per chunk."""
    ho, wo = h // 2, w // 2
    rowlen = w * c          # elems per input row
    outrow = wo * c         # elems per output row
    rp = b_loc * ho         # total output rows in this shard
    assert rowlen % f == 0
    n_seg = rowlen // f
    g = f // 2              # output elems per partition per chunk
    q = f // (2 * c)        # pixel-pairs per segment

    nc = bass.Bass()
    x = nc.declare_dram_parameter("x", [b_loc, h, w, c], F32, isOutput=False)
    y = nc.declare_dram_parameter("y", [b_loc, ho, wo, c], F32, isOutput=True)

    # [rp, parity(2), rowlen]: row-pairs across the whole shard (batch rows
    # are contiguous so (b h) flattens seamlessly).
    xv = x[:].rearrange("b h w c -> (b h) (w c)").rearrange(
        "(hp par) f -> hp par f", par=2
    )
    yv = y[:].rearrange("b h w c -> (b h) (w c)")  # [rp, outrow]

    # partition blocks over output rows
    blocks = []
    p0 = 0
    while p0 < rp:
        pn = min(128, rp - p0)
        blocks.append((p0, pn))
        p0 += pn

    with ExitStack() as ctx:
        tc = ctx.enter_context(tile.TileContext(nc))
        iop = ctx.enter_context(tc.tile_pool(name="io", bufs=3))
        epp = ctx.enter_context(tc.tile_pool(name="ex", bufs=2))
        tmp = ctx.enter_context(tc.tile_pool(name="tmp", bufs=2))
        outp = ctx.enter_context(tc.tile_pool(name="outp", bufs=1))
        out_ctr = [0]

        mul = mybir.AluOpType.mult
        add = mybir.AluOpType.add

        # prev-chunk state for the software-pipelined division tail:
        # (s1v, rv, n1v, dest-slice, q_l, g_l) of chunk k-1 is finished while
        # chunk k's product ops run, so every dependent pair (t->u->out) is
        # separated by an independent op and pays no DVE DRAIN bubble.
        prev = None

        def emit_tail(st, step):
            s1v_p, rv_p, n1v_p, dst, q_l, g_l = st[:6]
            pn_l = s1v_p.shape[0]
            if step == 0:
                t = tmp.tile([pn_l, g_l], F32, name="t", tag="t")
                st.append(t[:].rearrange("p (q c) -> p q c", q=q_l, c=c))
                nc.vector.tensor_tensor(st[6], s1v_p, rv_p, mul)
            elif step == 1:
                u = tmp.tile([pn_l, g_l], F32, name="u", tag="u")
                st.append(u[:].rearrange("p (q c) -> p q c", q=q_l, c=c))
                nc.vector.scalar_tensor_tensor(
                    st[7], st[6], 2.0, rv_p, mybir.AluOpType.subtract, mul
                )
            else:
                tag = f"outt{out_ctr[0] % 6}"
                out_ctr[0] += 1
                outt = outp.tile([pn_l, g_l], F32, name=tag, tag=tag)
                outtv = outt[:].rearrange("p (q c) -> p q c", q=q_l, c=c)
                nc.vector.scalar_tensor_tensor(outtv, n1v_p, -1.0, st[7], mul, mul)
                nc.sync.dma_start(dst, outt[:])

        for bi, (p0, pn) in enumerate(blocks):
            # split the very first chunk into quarter segments so the first
            # DVE op starts after a quarter-size DMA+exp (pipeline fill)
            if bi == 0 and f % (4 * 2 * c) == 0:
                seglens = [f // 4] * 4 + [f] * (n_seg - 1)
            else:
                seglens = [f] * n_seg
            off = 0
            for fl in seglens:
                ql = fl // (2 * c)
                gl = fl // 2
                xin = iop.tile([pn, 2 * f], F32, name="xin", tag="xin")
                xin3 = xin[:, 0:2 * fl].rearrange("p (par f) -> p par f", par=2)
                # issue input DMA from the ACT sequencer: the exp's WAR/RAW
                # edges become same-engine (no extra sem waits on the DMA)
                nc.scalar.dma_start(xin3, xv[p0:p0 + pn, :, off:off + fl])

                ex = epp.tile([pn, 2 * f], F32, name="ex", tag="ex")
                nc.scalar.activation(ex[:, 0:2 * fl], xin[:, 0:2 * fl],
                                     mybir.ActivationFunctionType.Exp)

                def quad(t):
                    v = t[:, 0:2 * fl].rearrange(
                        "p (half q two c) -> p half q two c",
                        half=2, q=ql, two=2, c=c,
                    )
                    return (v[:, 0, :, 0, :], v[:, 0, :, 1, :],
                            v[:, 1, :, 0, :], v[:, 1, :, 1, :])

                A, Bv, Cv, Dv = quad(xin)
                EA, EB, EC, ED = quad(ex)

                def t3(tag):
                    t = tmp.tile([pn, gl], F32, name=tag, tag=tag)
                    return t, t[:].rearrange("p (q c) -> p q c", q=ql, c=c)

                # s-sum first so the ACT Ln/Exp reciprocal seed overlaps the
                # product chain; accumulations distance-separated from their
                # producers to dodge the per-op DRAIN bubble.
                s1, s1v = t3("s1")
                nc.vector.tensor_tensor(s1v, EA, EB, add)
                n1, n1v = t3("n1")
                nc.vector.tensor_tensor(n1v, A, EA, mul)
                s2, s2v = t3("s2")
                nc.vector.tensor_tensor(s2v, EC, ED, add)
                n2, n2v = t3("n2")
                nc.vector.tensor_tensor(n2v, Bv, EB, mul)
                nc.vector.tensor_tensor(s1v, s1v, s2v, add)

                # 1/s: ACT seed r = exp(-ln(s)) (Ln+Exp share one table set;
                # keeps the ~6 cyc/elem iterative divide off DVE), then one
                # DVE Newton step (in the next chunk's tail) for fp32
                # accuracy:  u = (s*r - 2)*r = -r';  out = (n * -1)*u = n*r'
                lns, _ = t3("lns")
                nc.scalar.activation(lns[:], s1[:], mybir.ActivationFunctionType.Ln)
                r, rv = t3("r")
                nc.scalar.activation(r[:], lns[:], mybir.ActivationFunctionType.Exp,
                                     scale=-1.0)

                n3, n3v = t3("n3")
                nc.vector.tensor_tensor(n3v, Cv, EC, mul)
                if prev is not None:
                    emit_tail(prev, 0)
                n4, n4v = t3("n4")
                nc.vector.tensor_tensor(n4v, Dv, ED, mul)
                if prev is not None:
                    emit_tail(prev, 1)
                nc.vector.tensor_tensor(n1v, n1v, n2v, add)
                if prev is not None:
                    emit_tail(prev, 2)
                nc.vector.tensor_tensor(n3v, n3v, n4v, add)
                nc.vector.tensor_tensor(n1v, n1v, n3v, add)

                prev = [s1v, rv, n1v,
                        yv[p0:p0 + pn, off // 2:off // 2 + gl], ql, gl]
                off += fl

        for step in range(3):
            emit_tail(prev, step)

    return _legalize_waits(nc) if legalize else nc


def kernel(**inputs) -> np.ndarray:
    from concourse.bass_utils import run_bass_kernel_spmd

    x = inputs["x"]
    assert x.shape == (B, H, W, C) and x.dtype == np.float32
    nc = build_kernel()
    shards = x.reshape(N_CORES, B_LOC, H, W, C)
    in_maps = [{"x": np.ascontiguousarray(shards[i])} for i in range(N_CORES)]
    res = run_bass_kernel_spmd(nc, in_maps, list(range(N_CORES)))
    return np.concatenate([r["y"] for r in res.results], axis=0)


if __name__ == "__main__":
    # Small-shape CoreSim validation (no hardware).
    from concourse.bass_interp import CoreSim

    b_loc, h, w, c, f = 1, 8, 16, 128, 1024
    nc = build_kernel(b_loc, h, w, c, f, legalize=False)
    rng = np.random.default_rng(0)
    xs = rng.standard_normal((b_loc, h, w, c), dtype=np.float32)

    sim = CoreSim(nc)
    sim.tensor("x")[:] = xs
    sim.simulate()
    got = sim.tensor("y").copy()

    xd = xs.astype(np.float64)
    p = xd.reshape(b_loc, h // 2, 2, w // 2, 2, c).transpose(0, 1, 3, 2, 4, 5)
    p = p.reshape(b_loc, h // 2, w // 2, 4, c)
    e = np.exp(p - p.max(axis=3, keepdims=True))
    ref = (p * e).sum(axis=3) / e.sum(axis=3)
    err = np.abs(got - ref).max() / np.abs(ref).max()
    print("scale-rel err:", err, "max abs err:", np.abs(got - ref).max())
    assert err < 1e-5, "sim mismatch"
    print("SIM OK")



# revision 13
# speedup vs baseline: 1.1761x; 1.1761x over previous
"""AttMaxPool2D (2x2 softmax-attention pooling) Trainium2 Bass kernel.

Problem: x [16, 224, 224, 128] f32 NHWC -> out [16, 112, 112, 128]
  patches = 2x2 non-overlapping windows; out = sum(p * softmax(p, axis=window)).

Sharding: pure data parallel over batch: 8 cores x 2 examples each.

Per-core design (v2 -- DVE was the 97%-busy bottleneck in v1):
  * Quarter-row partitioning: the 224 output rows x 4 row-quarters = 896
    quarter-rows = 7 passes x 128 partitions, so every engine op runs with
    all 128 partitions busy (v1's 128+96 row blocks wasted 14% of DVE, since
    op cost depends only on free-dim length).
  * Work split across engines:
      ACT:    exp(x) over the input, then ln(S) and exp(-ln(S)) ~= 1/S
              (skip v1's Newton step; table accuracy ~1e-6 passes the gate)
      GpSimd: denominator sum tree S = sum of the 4 exps (2 ops: row-pair
              add on contiguous halves, then even+odd column add)
      DVE:    4 window products x*e^x (f32 in, bf16 out), numerator adds in
              bf16 (packed 2-byte operands hit the 2x_1p DVE fast path),
              final out = N * (1/S) in f32
  * Software pipeline with depth-2 deferral of the ln/recip/final-multiply
    chain so no engine head-of-line blocks on another chunk's dependencies.

Numerics: bf16 numerator gives rel err ~7.6e-3 vs the 2e-2 gate (validated
offline on the exact problem input against an fp64 reference).
"""

import os
from contextlib import ExitStack

import numpy as np

import concourse.bass as bass
import concourse.mybir as mybir
import concourse.tile as tile

F32 = mybir.dt.float32
BF16 = mybir.dt.bfloat16

# Full problem shape (hardcoded per contract).
B, H, W, C = 16, 224, 224, 128
N_CORES = 8
B_LOC = B // N_CORES
NQ = 4  # row quarters


def _legalize_waits(nc, max_waits=1):
    """This walrus build's ISA structs accept a single sync-wait command per
    instruction, but Tile's wait emission (not transitively minimal) can leave
    2+ waits.  Two-step fix, semantics-preserving:
      1. prune a wait when it is provably dominated through a kept wait
         (some instruction on the kept wait's engine proc, at/before the kept
         wait value, itself directly waits on the dropped semaphore at >= the
         dropped value);
      2. hoist any remaining extras onto same-engine NoOp instructions
         inserted immediately before (sequencer program order preserves the
         blocking semantics)."""
    import bass_rust
    from concourse.tile_scheduler import PROC_NAME_TO_IDX

    f = nc.m.functions[0]
    insts = [i for b in f.blocks for i in b.instructions]

    def pidx(ant_name):
        return PROC_NAME_TO_IDX[ant_name.rsplit("_", 1)[0]]

    by_proc = {}
    for i in insts:
        p = getattr(i, "bass_scheduled_proc", None)
        t = getattr(i, "bass_scheduled_tick", None)
        if p is None or t is None:
            continue
        by_proc.setdefault(p, []).append((t, i))
    for v in by_proc.values():
        v.sort(key=lambda x: x[0])

    def direct_waits(j):
        si = j.sync_info
        out = {}
        for w in si.on_wait if si else []:
            k = pidx(w.ant_name)
            out[k] = max(out.get(k, -1), w.wait_value)
        return out

    engine_procs = {v for k, v in PROC_NAME_TO_IDX.items()
                    if not k.startswith(("DMAHW", "DMASW", "Collectives"))}

    nop_ctr = [0]
    for b in f.blocks:
        new_insts = []
        for i in b.instructions:
            si = i.sync_info
            if not si or len(si.on_wait) <= max_waits:
                new_insts.append(i)
                continue
            # dedupe per-sem (keep max value)
            best = {}
            for w in si.on_wait:
                k = (w.sync_type, w.id)
                if k not in best or w.wait_value > best[k].wait_value:
                    best[k] = w
            kept = list(best.values())
            # drop same-proc self-waits: an engine instruction waiting on its
            # own proc's semaphore for a tick strictly below its own scheduled
            # tick is guaranteed by program order (the engine runs serially);
            # keeping it only stalls on the ~1us deferred sem-write of the
            # predecessor.
            own_p = getattr(i, "bass_scheduled_proc", None)
            own_t = getattr(i, "bass_scheduled_tick", None)
            if own_p is not None and own_t is not None and i.opcode != "DMACopy":
                kept = [w for w in kept
                        if not (pidx(w.ant_name) == own_p
                                and w.wait_value < own_t)]
            # step 1: transitive pruning
            for wd in list(kept):
                if len(kept) <= max_waits:
                    break
                wd_p, wd_v = pidx(wd.ant_name), wd.wait_value
                ok = False
                for via in kept:
                    if via is wd:
                        continue
                    via_p, via_v = pidx(via.ant_name), via.wait_value
                    if via_p not in engine_procs:
                        continue
                    for t, j in by_proc.get(via_p, []):
                        if t > via_v:
                            break
                        if direct_waits(j).get(wd_p, -1) >= wd_v:
                            ok = True
                            break
                    if ok:
                        break
                if ok:
                    kept.remove(wd)
            # step 2: hoist extras onto preceding same-engine NoOps
            while len(kept) > max_waits:
                w = kept.pop(0)
                nop = mybir.InstNoOp(name=f"I-waitnop-{nop_ctr[0]}", ins=[], outs=[])
                nop_ctr[0] += 1
                nop.engine = i.engine
                nop.sync_info = bass_rust.SyncInfo(on_wait=[w], on_update=[])
                new_insts.append(nop)
            si.on_wait = kept
            new_insts.append(i)
        b.instructions = new_insts
    return nc


def build_kernel(b_loc=B_LOC, h=H, w=W, c=C, fl=1792, bf16_n=True,
                 legalize=True):
    """Emit the per-core kernel.

    fl = input-row-quarter segment length (elems per parity row) per chunk.
    Layout: output quarter-rows qr = rp*NQ (rp = b_loc*h/2 row-pairs), mapped
    to partitions as p = pr*NQ + p4 with rp = k*(128//NQ) + pr, k passes.
    """
    ho, wo = h // 2, w // 2
    rowlen = w * c            # elems per input row (28672)
    outrow = wo * c           # elems per output row (14336)
    rp = b_loc * ho           # row-pairs in this shard (224)
    q_in = rowlen // NQ       # input quarter len per parity row (7168)
    q_out = outrow // NQ      # output quarter len (3584)
    assert (rp * NQ) % 128 == 0
    n_k = rp * NQ // 128      # passes (7)
    n_pr = 128 // NQ          # 32
    assert q_in % fl == 0
    n_j = q_in // fl          # j-chunks per quarter
    gl = fl // 2              # output elems per partition per chunk
    ql = fl // (2 * c)        # pixel-pairs per chunk
    nd = BF16 if bf16_n else F32

    nc = bass.Bass()
    x = nc.declare_dram_parameter("x", [b_loc, h, w, c], F32, isOutput=False)
    y = nc.declare_dram_parameter("y", [b_loc, ho, wo, c], F32, isOutput=True)

    # [128, n_k, 2(par), q_in]: partition = (pr, p4); row-pair = k*n_pr + pr.
    xq = (
        x[:]
        .rearrange("b h w c -> (b h) (w c)")
        .rearrange("(hp par) f -> hp par f", par=2)
        .rearrange("(k pr) par (p4 j) -> pr p4 k par j", pr=n_pr, p4=NQ)
    )  # [n_pr, NQ, n_k, 2, q_in]; partition p = pr*NQ + p4
    # [128, n_k, q_out]
    yq = (
        y[:]
        .rearrange("b h w c -> (b h) (w c)")
        .rearrange("(k pr) (p4 j) -> pr p4 k j", pr=n_pr, p4=NQ)
    )  # [n_pr, NQ, n_k, q_out]

    mul = mybir.AluOpType.mult
    add = mybir.AluOpType.add

    with ExitStack() as ctx:
        tc = ctx.enter_context(tile.TileContext(nc))
        iop = ctx.enter_context(tc.tile_pool(name="io", bufs=3))
        epp = ctx.enter_context(tc.tile_pool(name="ex", bufs=3))
        tmp = ctx.enter_context(tc.tile_pool(name="tmp", bufs=2))
        dfp = ctx.enter_context(tc.tile_pool(name="dfp", bufs=3))
        outp = ctx.enter_context(tc.tile_pool(name="outp", bufs=4))
        out_ctr = [0]

        # deferred per-chunk tails, emitted 1 and 2 chunks later:
        #   stage A (i+1 on ACT): lns = Ln(S); r = Exp(-lns)
        #   stage B (i+2 on DVE): out = N * r; dma out
        pend_a = []  # (s_tile, gl_l, dst, N_view, r_tile placeholder)
        pend_b = []

        def emit_stage_a(st):
            s_t, gl_l, dst, n_v = st
            lns = tmp.tile([128, gl_l], F32, name="lns", tag="lns")
            nc.scalar.activation(lns[:], s_t[:], mybir.ActivationFunctionType.Ln)
            r = dfp.tile([128, gl_l], F32, name="r", tag="r")
            nc.scalar.activation(r[:], lns[:], mybir.ActivationFunctionType.Exp,
                                 scale=-1.0)
            pend_b.append((n_v, r, gl_l, dst))

        def emit_stage_b(st):
            n_v, r, gl_l, dst = st
            tag = f"outt{out_ctr[0] % 4}"
            out_ctr[0] += 1
            outt = outp.tile([128, gl_l], F32, name=tag, tag=tag)
            nc.vector.tensor_tensor(outt[:], n_v, r[:], mul)
            nc.sync.dma_start(dst, outt[:])

        chunks = [(k, j0) for k in range(n_k) for j0 in range(0, q_in, fl)]
        for ci, (k, j0) in enumerate(chunks):
            # DMA APs are limited to 3 dims and tiles want a single DMA
            # writer: one tile + transfer per parity row, each
            # [pr, p4, j] <- [pr, p4, j].
            xins, exs = [], []
            for par in (0, 1):
                xin = iop.tile([128, fl], F32, name=f"xin{par}",
                               tag=f"xin{par}")
                nc.sync.dma_start(xin[:], xq[:, :, k, par, j0:j0 + fl])
                xins.append(xin)
            for par in (0, 1):
                ex = epp.tile([128, fl], F32, name=f"ex{par}", tag=f"ex{par}")
                nc.scalar.activation(ex[:], xins[par][:],
                                     mybir.ActivationFunctionType.Exp)
                exs.append(ex)
            # chunk ci-1's ln/recip go behind chunk ci's exp on ACT so the
            # ACT sequencer never stalls waiting for ci-1's GpSimd sum.
            if pend_a:
                emit_stage_a(pend_a.pop(0))

            def pair(t):
                v = t[:].rearrange("p (q two c) -> p q two c",
                                   q=ql, two=2, c=c)
                return v[:, :, 0, :], v[:, :, 1, :]

            A, Bv = pair(xins[0])
            Cv, Dv = pair(xins[1])
            EA, EB = pair(exs[0])
            EC, ED = pair(exs[1])

            # GpSimd: S = (EA+EC) + (EB+ED).  First add is the two parity
            # rows elementwise (contiguous fl-long tiles), second folds
            # even+odd columns (stride-c views).
            srow = tmp.tile([128, fl], F32, name="srow", tag="srow")
            nc.gpsimd.tensor_tensor(srow[:], exs[0][:], exs[1][:], add)
            sv = srow[:].rearrange("p (q two c) -> p q two c", q=ql, two=2, c=c)
            s = tmp.tile([128, gl], F32, name="s", tag="s")
            s3 = s[:].rearrange("p (q c) -> p q c", q=ql, c=c)
            nc.gpsimd.tensor_tensor(s3, sv[:, :, 0, :], sv[:, :, 1, :], add)

            # DVE: window products (bf16 out) + numerator adds (bf16 2x).
            def t3(tag, dt, pool=tmp):
                t = pool.tile([128, gl], dt, name=tag, tag=tag)
                return t, t[:].rearrange("p (q c) -> p q c", q=ql, c=c)

            n1, n1v = t3("n1", nd)
            nc.vector.tensor_tensor(n1v, A, EA, mul)
            n2, n2v = t3("n2", nd)
            nc.vector.tensor_tensor(n2v, Bv, EB, mul)
            if pend_b:
                emit_stage_b(pend_b.pop(0))
            n3, n3v = t3("n3", nd)
            nc.vector.tensor_tensor(n3v, Cv, EC, mul)
            n4, n4v = t3("n4", nd)
            nc.vector.tensor_tensor(n4v, Dv, ED, mul)
            n12, _ = t3("n12", nd)
            nc.vector.tensor_tensor(n12[:], n1[:], n2[:], add)
            n34, _ = t3("n34", nd)
            nc.vector.tensor_tensor(n34[:], n3[:], n4[:], add)
            ntot, _ = t3("ntot", nd, pool=dfp)
            nc.vector.tensor_tensor(ntot[:], n12[:], n34[:], add)

            pend_a.append((s, gl,
                           yq[:, :, k, j0 // 2:j0 // 2 + gl], ntot[:]))

        while pend_a or pend_b:
            if pend_a:
                emit_stage_a(pend_a.pop(0))
            if pend_b:
                emit_stage_b(pend_b.pop(0))

    return _legalize_waits(nc) if legalize else nc


def kernel(**inputs) -> np.ndarray:
    from concourse.bass_utils import run_bass_kernel_spmd

    x = inputs["x"]
    assert x.shape == (B, H, W, C) and x.dtype == np.float32
    nc = build_kernel()
    shards = x.reshape(N_CORES, B_LOC, H, W, C)
    in_maps = [{"x": np.ascontiguousarray(shards[i])} for i in range(N_CORES)]
    res = run_bass_kernel_spmd(nc, in_maps, list(range(N_CORES)))
    return np.concatenate([r["y"] for r in res.results], axis=0)


if __name__ == "__main__":
    # Small-shape CoreSim validation (no hardware).
    from concourse.bass_interp import CoreSim

    b_loc, h, w, c, fl = 1, 64, 32, 128, 512
    nc = build_kernel(b_loc, h, w, c, fl, legalize=False)
    rng = np.random.default_rng(0)
    xs = rng.standard_normal((b_loc, h, w, c), dtype=np.float32)

    sim = CoreSim(nc)
    sim.tensor("x")[:] = xs
    sim.simulate()
    got = sim.tensor("y").copy()

    xd = xs.astype(np.float64)
    p = xd.reshape(b_loc, h // 2, 2, w // 2, 2, c).transpose(0, 1, 3, 2, 4, 5)
    p = p.reshape(b_loc, h // 2, w // 2, 4, c)
    e = np.exp(p - p.max(axis=3, keepdims=True))
    ref = (p * e).sum(axis=3) / e.sum(axis=3)
    err = np.abs(got - ref).max() / np.abs(ref).max()
    print("scale-rel err:", err, "max abs err:", np.abs(got - ref).max())
    assert err < 2e-2, "sim mismatch"
    print("SIM OK (bf16 path)" if err > 1e-5 else "SIM OK")
